# revision 43
# baseline (speedup 1.0000x reference)
"""Trainium2 Bass kernel for nn_AttentionModel (patch-transformer + MSE loss).

Model (per batch element b of B=32):
    x[b] : [L=32768] --instance-norm--> patches [T=1024, PS=32]
    h    = patches @ W_proj + b_proj                  [T, 256]
    qkv  = h @ W_qkv + b_qkv ;  q,k,v = split(qkv)    [T, 256] each
    attn = softmax(causal(q k^T / 16))                [T, T]
    out  = (attn @ v) @ W_out + b_out                 [T, 256]
    pred = out @ W_head + b_head                      [T, PS]
    loss = mean((pred[:, :-1] - patches[:, 1:])**2)   scalar

Sharding: data-parallel over batch, 4 batch elements per core x 8 cores.
Each core computes a partial sum-of-squares; host combines.

v2.1 design (numpy prototype: proto_v2.py):
  - logits factor through patch space: scores[t,s] = x_t^T A x_s + g_s with
    A = Wq_eff Wk_eff^T [32,32]; the per-t bias constant is dropped
    (softmax-invariant), the per-s part g_s = (Wk b_q)^T x_s rides as a
    per-partition bias of the exp.
  - X = patches^T [32, T] bf16 normalized; Y = A^T X once [32, T];
    scores^T[s,t] computed per s-tile j as ONE wide matmul (N<=1024 bf16),
    causally trimmed.
  - exp is split between ScalarE (ACT exp, per-partition bias) and DVE
    (int16 bit-trick exp writing bf16 bits; its systematic error cancels
    in the softmax ratio -- validated 4.5e-6 rel err in proto).
  - PV runs TRANSPOSED: puT[tc] = sum_j et(j,tc)^T vw_j with
    vw = [X^T M_v + 1 c_v^T | ones], so the softmax denominator lands
    per-PARTITION (puT col 32) and normalize+loss is all per-partition:
    reciprocal [128,8] -> fused (puT*r - xn2) scalar_tensor_tensor ->
    Square+accum loss.
  - the shifted target patches xn2 (= patch[t+1] token-major) come from a
    second strided DMA of x (no transposes, no partition shifts).
  - NOTE row-tiled matmuls (tile_position with row base != 0) pass
    compile+sim but hang this HW stack when >1 issue back-to-back; only
    plain/col tiling is used.
"""

import math
import os

import numpy as np

import concourse.bass as bass
import concourse.mybir as mybir
import concourse.tile as tile
from concourse.bass_utils import run_bass_kernel_spmd
from concourse.masks import make_identity, make_upper_triangular
from concourse.vector_clock import ScopedClock

F32 = mybir.dt.float32
BF16 = mybir.dt.bfloat16
I16 = mybir.dt.int16
AX = mybir.AxisListType
ALU = mybir.AluOpType
AF = mybir.ActivationFunctionType

N_CORES = 8
B = 32
L = 32768
PS = 32
D = 256
T = L // PS  # 1024
BPC = B // N_CORES  # batch elements per core = 4
SCALE = 1.0 / math.sqrt(D)  # 1/16
K1 = 128.0 / math.log(2.0)  # bf16 exponent scale for bit-trick exp
B0 = 127.0 * 128.0 - 0.0579 * 128.0  # bias incl. mantissa-centering corr

# j-tiles whose exp runs on DVE (bit-trick); rest on ScalarE ACT exp.
DVE_EXP = {0, 1}


class SplitDrainTileContext(tile.TileContext):
    """TileContext whose final drain splits sem waits across multiple drain
    instructions -- this walrus rejects >1 sync wait per instruction."""

    def _drain_and_barrier(self, tick_clock, wait_clock):
        probe = mybir.InstDrain(name=f"I-{self.nc.next_id()}", ins=[], outs=[])
        probe.engine = mybir.EngineType.SP
        wait_clock.add_sem_waits(probe, ScopedClock({None: tick_clock.global_clock}))
        waits = list(probe.sync_info.on_wait) if probe.sync_info else []
        assert self.sems is not None
        handles = {h.num: h for h in self.sems.allocated().values()}
        if not waits:
            self.nc.sync.drain()
        for w in waits:
            d = self.nc.sync.drain()
            d.wait_op(handles[w.id], w.wait_value, "sem-ge", check=False)
        self.nc.all_engine_barrier()
        popped = self.nc._tile_sem_poison_stack.pop()
        assert popped is self._sem_poison
        self.nc.clear_and_free_semaphores(list(self.sems.allocated().values()))
        self.nc.all_engine_barrier()


def split_excess_waits(nc, max_waits=1):
    """This walrus rejects instructions carrying more than one sync wait.
    Hoist extra waits onto the immediately preceding same-engine
    instruction when that instruction signals nothing, else insert a
    wait-only drain."""
    for f in nc.m.functions:
        for blk in f.blocks:
            insts = list(blk.instructions)
            out = []
            prev_by_engine = {}
            changed = False
            for inst in insts:
                si = inst.sync_info
                waits = list(si.on_wait) if si else []
                if len(waits) > max_waits:
                    changed = True
                    extra, keep = waits[:-max_waits], waits[-max_waits:]
                    remaining = []
                    prev = prev_by_engine.get(str(inst.engine))
                    for w in extra:
                        psi = prev.sync_info if prev is not None else None
                        if prev is not None and (
                            psi is None
                            or (len(psi.on_wait) == 0 and len(psi.on_update) == 0)
                        ):
                            prev.sync_info = mybir.SyncInfo(on_wait=[w], on_update=[])
                            prev = None  # one hoist per predecessor
                        else:
                            remaining.append(w)
                    for w in remaining:
                        dr = mybir.InstDrain(name=f"I-{nc.next_id()}", ins=[], outs=[])
                        dr.engine = inst.engine
                        dr.sync_info = mybir.SyncInfo(on_wait=[w], on_update=[])
                        out.append(dr)
                    inst.sync_info = mybir.SyncInfo(
                        on_wait=keep, on_update=list(si.on_update)
                    )
                out.append(inst)
                prev_by_engine[str(inst.engine)] = inst
            if changed:
                blk.instructions = out


def dedupe_ldweights(nc):
    """Drop an InstLdweights whose operand AP AND tile_position are
    byte-identical to the immediately preceding PE instruction's
    InstLdweights -- the stationary operand is still loaded. Only legal
    when the elided load carries no sync actions."""
    for f in nc.m.functions:
        for blk in f.blocks:
            insts = list(blk.instructions)
            out = []
            last_pe_ldw_key = None
            changed = False
            for inst in insts:
                if str(inst.engine) != "EngineType.PE":
                    out.append(inst)
                    continue
                tname = type(inst).__name__
                if tname == "InstLdweights":
                    si = inst.sync_info
                    has_sync = si and (len(si.on_wait) or len(si.on_update))
                    try:
                        key = (
                            str(inst.ins[0]),
                            str(getattr(inst, "tile_position", None)),
                        )
                    except Exception:
                        key = None
                    if key is not None and key == last_pe_ldw_key and not has_sync:
                        changed = True
                        continue  # elide duplicate load
                    last_pe_ldw_key = key
                    out.append(inst)
                else:
                    if tname == "InstMatmult":
                        # transpose-mode matmuls reload the array themselves
                        if getattr(inst, "is_transpose", None):
                            last_pe_ldw_key = None
                    else:
                        last_pe_ldw_key = None
                    out.append(inst)
            if changed:
                blk.instructions = out


def build_program(postprocess=True, has_bias=False):
    nc = bass.Bass("TRN2", target_bir_lowering=False, debug=False, num_devices=N_CORES)

    xt_d = nc.dram_tensor("x_t", [BPC * PS, T], BF16, kind="ExternalInput")
    x2_d = nc.dram_tensor("x2_h", [BPC * 128, 8 * PS], F32, kind="ExternalInput")
    a_d = nc.dram_tensor("a_mat", [PS, PS], BF16, kind="ExternalInput")
    mvg_d = nc.dram_tensor("mvg", [PS, PS + 2], BF16, kind="ExternalInput")
    row_d = nc.dram_tensor("rows", [1, 2 * PS], F32, kind="ExternalInput")
    out_d = nc.dram_tensor("loss_partial", [1, 1], F32, kind="ExternalOutput")

    from contextlib import ExitStack

    with SplitDrainTileContext(nc) as tc, ExitStack() as ctx:
        cpool = ctx.enter_context(tc.tile_pool(name="consts", bufs=1))
        # PSUM pools (8 banks x 2KB/partition): pscore 3x2 banks + ppu 2x1
        pscore = ctx.enter_context(tc.tile_pool(name="pscore", bufs=3, space="PSUM"))
        ppu = ctx.enter_context(tc.tile_pool(name="ppu", bufs=2, space="PSUM"))
        # SBUF pools
        xpool = ctx.enter_context(tc.tile_pool(name="xc", bufs=4))
        x2pool = ctx.enter_context(tc.tile_pool(name="xc2", bufs=4))
        qpool = ctx.enter_context(tc.tile_pool(name="xnt", bufs=4))
        ypool = ctx.enter_context(tc.tile_pool(name="y", bufs=3))
        vpool = ctx.enter_context(tc.tile_pool(name="vw", bufs=3))
        epool = ctx.enter_context(tc.tile_pool(name="et", bufs=3))
        spool = ctx.enter_context(tc.tile_pool(name="small", bufs=6))
        lpool = ctx.enter_context(tc.tile_pool(name="loss", bufs=4))

        # ---- constants ----
        ident_f = cpool.tile([128, 128], F32)
        make_identity(nc, ident_f[:])
        triu_b = cpool.tile([128, 128], BF16)
        make_upper_triangular(nc, triu_b[:], val=1.0, diag=True)
        ones_col = cpool.tile([128, 1], F32)
        nc.vector.memset(ones_col[:], 1.0)
        ones_row = cpool.tile([1, 128], F32)
        nc.vector.memset(ones_row[:], 1.0)

        a_mat = cpool.tile([PS, PS], BF16)
        nc.gpsimd.dma_start(a_mat[:], a_d.ap()[:])
        mvg = cpool.tile([PS, PS + 2], BF16)
        nc.gpsimd.dma_start(mvg[:], mvg_d.ap()[:])
        rows = cpool.tile([1, 2 * PS], F32)  # [1^T M_v | c_v]
        nc.gpsimd.dma_start(rows[:], row_d.ap()[:])

        lp = cpool.tile([128, BPC], F32)  # per-batch loss partials
        # last-partition mask: zeroes the t=1023 dd column (excluded by
        # pred[:, :-1]; its xn2 is garbage)
        lmask = cpool.tile([128, 1], F32)
        nc.vector.tensor_scalar(
            out=lmask[:],
            in0=ident_f[:, 127:128],
            scalar1=-1.0,
            scalar2=1.0,
            op0=ALU.mult,
            op1=ALU.add,
        )

        # PE warm-up: HAM holds PE at 1.2 GHz until ~3.4us sustained
        # activity; burn dummy matmuls while the DMAs + stats run.
        warm_ps = pscore.tile([128, 1024], F32, tag="sc")
        for _ in range(60):
            nc.tensor.matmul(
                warm_ps[:, 0:128], triu_b[:], triu_b[:], start=True, stop=True
            )

        # ---- load x_T (patches^T bf16) and shifted target patches ----
        xnt_t = []
        xc2 = []
        for b in range(BPC):
            t_ = qpool.tile([PS, T], BF16, name=f"xnt{b}", tag="xnt")
            nc.sync.dma_start(t_[:], xt_d.ap()[b * PS : (b + 1) * PS, :])
            xnt_t.append(t_)
            t2 = x2pool.tile([128, 8 * PS], F32, name=f"xc2{b}", tag="xc2")
            nc.sync.dma_start(t2[:], x2_d.ap()[b * 128 : (b + 1) * 128, :])
            xc2.append(t2)

        # ---- stats: sums over x2_h (= x minus patch 0, plus zeros) with a
        # patch-0 correction via a second accumulating matmul ----
        sums = spool.tile([128, 2 * BPC], F32, name="sums")
        sqscr = spool.tile([128, 8 * PS], F32, name="sqscr")
        p0sq = spool.tile([PS, 2 * BPC], F32, name="p0sq")
        for b in range(BPC):
            nc.vector.tensor_reduce(
                sums[:, b : b + 1], xc2[b][:], axis=AX.X, op=ALU.add
            )
            nc.scalar.activation(
                sqscr[:],
                xc2[b][:],
                AF.Square,
                accum_out=sums[:, BPC + b : BPC + b + 1],
            )
            p0 = xnt_t[b][:, 0:1]
            nc.vector.tensor_copy(p0sq[:, b : b + 1], p0)
            nc.vector.tensor_tensor(
                out=p0sq[:, BPC + b : BPC + b + 1], in0=p0, in1=p0, op=ALU.mult
            )
        tot_ps = ppu.tile([1, 2 * BPC], F32, tag="pu")
        nc.tensor.matmul(tot_ps[:], ones_col[:], sums[:], start=True, stop=False)
        nc.tensor.matmul(
            tot_ps[:], ones_col[0:PS, :], p0sq[:], start=False, stop=True
        )
        tot = spool.tile([1, 2 * BPC], F32, name="tot")
        nc.vector.tensor_copy(tot[:], tot_ps[:])

        # vectorized stats chain over the 4 batches
        sc = spool.tile([1, 8 * BPC], F32, name="sc")
        mean = sc[:, 0:BPC]
        sm = sc[:, BPC : 2 * BPC]
        varr = sc[:, 2 * BPC : 3 * BPC]
        lnv = sc[:, 3 * BPC : 4 * BPC]
        stde = sc[:, 4 * BPC : 5 * BPC]
        rstd = sc[:, 5 * BPC : 6 * BPC]
        r2 = sc[:, 6 * BPC : 7 * BPC]
        r2m = sc[:, 7 * BPC : 8 * BPC]
        nc.scalar.mul(mean, tot[:, 0:BPC], 1.0 / L)
        nc.vector.tensor_tensor(out=sm, in0=tot[:, 0:BPC], in1=mean, op=ALU.mult)
        nc.vector.tensor_tensor(
            out=varr, in0=tot[:, BPC : 2 * BPC], in1=sm, op=ALU.subtract
        )
        nc.scalar.activation(lnv, varr, AF.Ln, scale=1.0 / (L - 1))
        nc.scalar.activation(stde, lnv, AF.Exp, scale=0.5)  # std
        nc.vector.tensor_scalar_add(stde, stde, 1e-5)
        nc.vector.reciprocal(rstd, stde)  # r
        nc.vector.tensor_tensor(out=r2, in0=rstd, in1=rstd, op=ALU.mult)
        nc.vector.tensor_tensor(out=r2m, in0=r2, in1=mean, op=ALU.mult)
        # broadcast rows: [r^2*K1*SCALE (dve exp mul), r^2*SCALE (act exp
        # scale), -r^2 m*K1*SCALE (dve bias mul), -r^2 m*SCALE (act bias mul)]
        scv = spool.tile([1, 7 * BPC], F32, name="scv")
        nc.vector.tensor_scalar_mul(scv[:, 0:BPC], r2, K1 * SCALE)
        nc.vector.tensor_scalar_mul(scv[:, BPC : 2 * BPC], r2, SCALE)
        nc.vector.tensor_scalar_mul(scv[:, 2 * BPC : 3 * BPC], r2m, -K1 * SCALE)
        nc.vector.tensor_scalar_mul(scv[:, 3 * BPC : 4 * BPC], r2m, -SCALE)
        nc.vector.tensor_scalar_mul(scv[:, 4 * BPC : 5 * BPC], rstd, K1 * SCALE)
        nc.vector.tensor_scalar_mul(scv[:, 5 * BPC : 6 * BPC], rstd, SCALE)
        nc.vector.tensor_copy(scv[:, 6 * BPC : 7 * BPC], r2)
        bc_ps = ppu.tile([128, 7 * BPC], F32, tag="pu")
        nc.tensor.matmul(bc_ps[:], ones_row[:], scv[:], start=True, stop=True)
        bc = spool.tile([128, 7 * BPC], F32, name="bc")
        nc.vector.tensor_copy(bc[:], bc_ps[:])
        # c2row[b] = m_b*(1 - 1^T M_v) + c_v*std_b, replicated to all
        # partitions via a rank-1 matmul
        c2row1 = spool.tile([1, BPC * PS], F32, name="c2row1")
        negm = spool.tile([1, BPC], F32, name="negm")
        nc.vector.tensor_scalar_mul(negm[:], mean, -1.0)
        for b in range(BPC):
            seg = c2row1[:, b * PS : (b + 1) * PS]
            nc.vector.tensor_scalar(
                out=seg,
                in0=rows[:, 0:PS],
                scalar1=negm[:, b : b + 1],
                scalar2=mean[:, b : b + 1],
                op0=ALU.mult,
                op1=ALU.add,
            )
            nc.vector.scalar_tensor_tensor(
                out=seg,
                in0=rows[:, PS : 2 * PS],
                scalar=stde[:, b : b + 1],
                in1=seg,
                op0=ALU.mult,
                op1=ALU.add,
            )
        c2_ps = ppu.tile([128, BPC * PS], F32, tag="pu")
        nc.tensor.matmul(c2_ps[:], ones_row[:], c2row1[:], start=True, stop=True)
        c2rep = spool.tile([128, BPC * PS], F32, name="c2rep")
        nc.vector.tensor_copy(c2rep[:], c2_ps[:])

        ddscr = spool.tile([128, 8 * PS], BF16, name="ddscr")  # Square scratch

        state = {}

        def stage_ty(b):
            xnt = xnt_t[b]
            # Y = A^T X_raw [32, 1024]
            yb = ypool.tile([PS, T], BF16, name=f"y{b}", tag="y")
            y_ps = pscore.tile([128, 1024], F32, tag="sc", name=f"yp{b}")
            for h in range(2):
                nc.tensor.matmul(
                    y_ps[0:PS, 512 * h : 512 * (h + 1)],
                    a_mat[:],
                    xnt[:, 512 * h : 512 * (h + 1)],
                    start=True,
                    stop=True,
                )
            nc.scalar.copy(yb[:, 0:512], y_ps[0:PS, 0:512])
            nc.vector.tensor_copy(yb[:, 512:1024], y_ps[0:PS, 512:1024])
            vw = vpool.tile([128, 8 * (PS + 1)], BF16, name=f"vw{b}", tag="vw")
            nc.gpsimd.memset(
                vw[:].rearrange("u (j e) -> u j e", e=PS + 1)[:, :, PS : PS + 1],
                1.0,
            )
            bias_d = spool.tile([128, 8], F32, name=f"bd{b}", tag="biasd")
            bias_s = spool.tile([128, 8], F32, name=f"bs{b}", tag="biass")
            state[b] = dict(
                xnt=xnt, yb=yb, vw=vw, bias_d=bias_d, bias_s=bias_s
            )

        def stage_scores(b, pv_prev=None):
            st = state[b]
            xnt, yb, vw = st["xnt"], st["yb"], st["vw"]
            vwg_ps = ppu.tile([128, 8 * (PS + 2)], F32, tag="pu", name=f"vwg{b}")
            bias_d, bias_s = st["bias_d"], st["bias_s"]
            # (bias tiles are filled by emit_vw_evac below)
            et = epool.tile([128, 9216], BF16, name=f"et{b}", tag="et")
            st.update(et=et)
            score_tiles = {}

            def emit_scores(j):
                lhs = xnt[:, 128 * j : 128 * (j + 1)]
                nc.tensor.matmul(
                    vwg_ps[:, j * (PS + 2) : (j + 1) * (PS + 2)],
                    lhs,
                    mvg[:],
                    start=True,
                    stop=True,
                )
                s_ps = pscore.tile([128, 1024], F32, tag="sc", name=f"s{b}_{j}")
                score_tiles[j] = s_ps
                for c in range(2):
                    lo = max(512 * c, 128 * j)
                    hi = 512 * (c + 1)
                    if lo >= hi:
                        continue
                    nc.tensor.matmul(
                        s_ps[:, lo:hi], lhs, yb[:, lo:hi], start=True, stop=True
                    )

            def emit_vw_evac(half):
                # cast data cols to bf16; exp bias tiles from the gm/gk cols
                j0 = 4 * half
                pv_v = vwg_ps[
                    :, j0 * (PS + 2) : (j0 + 4) * (PS + 2)
                ].rearrange("u (j e) -> u j e", e=PS + 2)
                vw_v = vw[
                    :, j0 * (PS + 1) : (j0 + 4) * (PS + 1)
                ].rearrange("u (j e) -> u j e", e=PS + 1)
                nc.vector.tensor_copy(vw_v[:, :, 0:PS], pv_v[:, :, 0:PS])
                gm = pv_v[:, :, PS : PS + 1].rearrange("u j o -> u (j o)")
                nc.vector.tensor_scalar(
                    out=bias_d[:, j0 : j0 + 4],
                    in0=gm,
                    scalar1=bc[:, 2 * BPC + b : 2 * BPC + b + 1],
                    scalar2=B0,
                    op0=ALU.mult,
                    op1=ALU.add,
                )
                nc.vector.tensor_scalar(
                    out=bias_s[:, j0 : j0 + 4],
                    in0=gm,
                    scalar1=bc[:, 3 * BPC + b : 3 * BPC + b + 1],
                    scalar2=None,
                    op0=ALU.mult,
                )
                if has_bias:
                    gk = pv_v[:, :, PS + 1 : PS + 2].rearrange("u j o -> u (j o)")
                    nc.vector.scalar_tensor_tensor(
                        out=bias_d[:, j0 : j0 + 4],
                        in0=gk,
                        scalar=bc[:, 4 * BPC + b : 4 * BPC + b + 1],
                        in1=bias_d[:, j0 : j0 + 4],
                        op0=ALU.mult,
                        op1=ALU.add,
                    )
                    nc.vector.scalar_tensor_tensor(
                        out=bias_s[:, j0 : j0 + 4],
                        in0=gk,
                        scalar=bc[:, 5 * BPC + b : 5 * BPC + b + 1],
                        in1=bias_s[:, j0 : j0 + 4],
                        op0=ALU.mult,
                        op1=ALU.add,
                    )

            def emit_exp(j):
                s_ps = score_tiles.pop(j)
                src = s_ps[:, 128 * j : T]
                dst = et[:, 1024 * j + 128 * j : 1024 * (j + 1)]
                if j in DVE_EXP:
                    nc.vector.tensor_scalar(
                        out=dst.bitcast(I16),
                        in0=src,
                        scalar1=bc[:, b : b + 1],
                        scalar2=bias_d[:, j : j + 1],
                        op0=ALU.mult,
                        op1=ALU.add,
                    )
                else:
                    nc.scalar.activation(
                        dst,
                        src,
                        AF.Exp,
                        scale=bc[:, BPC + b : BPC + b + 1],
                        bias=bias_s[:, j : j + 1],
                    )

            def pv_group(tcn):
                if pv_prev is None:
                    return
                pst = state[pv_prev]
                pet, pvw = pst["et"], pst["vw"]
                for j in range(tcn + 1):
                    col = 1024 * j + 128 * tcn
                    nc.tensor.matmul(
                        pst["puT_ps"][:, tcn * (PS + 1) : (tcn + 1) * (PS + 1)],
                        pet[:, col : col + 128],
                        pvw[:, j * (PS + 1) : (j + 1) * (PS + 1)],
                        start=(j == 0),
                        stop=(j == tcn),
                    )

            if pv_prev is not None:
                state[pv_prev]["puT_ps"] = ppu.tile(
                    [128, 8 * (PS + 1)], F32, tag="pu", name=f"puT{pv_prev}"
                )
            for j in range(4):
                emit_scores(j)
                if j % 2 == 1:
                    pv_group(j - 1)
                    pv_group(j)
            emit_vw_evac(0)
            for j in range(4):
                emit_exp(j)
            for j in range(4, 8):
                emit_scores(j)
                if j % 2 == 1:
                    pv_group(j - 1)
                    pv_group(j)
            emit_vw_evac(1)
            for j in range(4, 8):
                emit_exp(j)
            # diag triu masks for THIS batch's et (stride 1152), split D/G
            vfull = et[:, 0 : 8 * 1152].rearrange("u (j w) -> u j w", w=1152)[
                :, :, 0:128
            ]
            tri4 = (
                triu_b[:]
                .rearrange("u (o w) -> u o w", o=1)
                .broadcast_to((128, 4, 128))
            )
            nc.vector.tensor_tensor(
                out=vfull[:, 0:4], in0=vfull[:, 0:4], in1=tri4, op=ALU.mult
            )
            nc.gpsimd.tensor_tensor(
                out=vfull[:, 4:8], in0=vfull[:, 4:8], in1=tri4, op=ALU.mult
            )

        def stage_pv_alone(b):
            st = state[b]
            et, vw = st["et"], st["vw"]
            st["puT_ps"] = ppu.tile(
                [128, 8 * (PS + 1)], F32, tag="pu", name=f"puT{b}"
            )
            for tcn in range(8):
                for j in range(tcn + 1):
                    col = 1024 * j + 128 * tcn
                    nc.tensor.matmul(
                        st["puT_ps"][:, tcn * (PS + 1) : (tcn + 1) * (PS + 1)],
                        et[:, col : col + 128],
                        vw[:, j * (PS + 1) : (j + 1) * (PS + 1)],
                        start=(j == 0),
                        stop=(j == tcn),
                    )

        def stage_epi(b):
            st = state[b]
            puT_ps = st["puT_ps"]
            # evacuate puT to SBUF immediately (frees the psum bank), then
            # normalize+subtract+square
            puT_sb = lpool.tile([128, 8 * (PS + 1)], F32, name=f"pus{b}", tag="pus")
            nc.scalar.copy(puT_sb[:], puT_ps[:])
            rcol = lpool.tile([128, 8], F32, name=f"rc{b}", tag="rc")
            nc.vector.reciprocal(
                rcol[:],
                puT_sb[:].rearrange("u (c e) -> u c e", e=PS + 1)[
                    :, :, PS : PS + 1
                ].rearrange("u c o -> u (c o)"),
            )
            rexp = lpool.tile([128, 8 * PS], BF16, name=f"rx{b}", tag="rx")
            nc.vector.tensor_copy(
                rexp[:].rearrange("u (c p) -> u c p", p=PS),
                rcol[:].rearrange("u (c o) -> u c o", o=1).broadcast_to(
                    (128, 8, PS)
                ),
            )
            dd = lpool.tile([128, 8 * PS], BF16, name=f"dd{b}", tag="dd")
            nc.vector.tensor_tensor(
                out=dd[:].rearrange("u (c p) -> u c p", p=PS),
                in0=puT_sb[:].rearrange("u (c e) -> u c e", e=PS + 1)[
                    :, :, 0:PS
                ],
                in1=rexp[:].rearrange("u (c p) -> u c p", p=PS),
                op=ALU.mult,
            )
            nc.gpsimd.tensor_tensor(
                out=dd[:], in0=dd[:], in1=xc2[b][:], op=ALU.subtract
            )
            nc.gpsimd.tensor_tensor(
                out=dd[:].rearrange("u (c p) -> u c p", p=PS),
                in0=dd[:].rearrange("u (c p) -> u c p", p=PS),
                in1=c2rep[:, b * PS : (b + 1) * PS]
                .rearrange("u (o p) -> u o p", o=1)
                .broadcast_to((128, 8, PS)),
                op=ALU.add,
            )
            nc.vector.tensor_scalar(  # exclude t=1023
                out=dd[:, 7 * PS : 8 * PS],
                in0=dd[:, 7 * PS : 8 * PS],
                scalar1=lmask[:],
                scalar2=None,
                op0=ALU.mult,
            )
            nc.scalar.activation(
                ddscr[:], dd[:], AF.Square, accum_out=lp[:, b : b + 1]
            )

        # software pipeline: batch b+1's transposes/Y/scores fill the PE
        # while batch b's exps drain and its PV+epilogue wait on them
        stage_ty(0)
        stage_scores(0)
        for b in range(1, BPC):
            stage_ty(b)
            stage_scores(b, pv_prev=b - 1)
            stage_epi(b - 1)
        stage_pv_alone(BPC - 1)
        stage_epi(BPC - 1)

        # ---- final: scale partials by r^2, then total ----
        nc.vector.tensor_tensor(
            out=lp[:], in0=lp[:], in1=bc[:, 6 * BPC : 7 * BPC], op=ALU.mult
        )
        lsum = spool.tile([128, 1], F32, name="lsum")
        nc.vector.tensor_reduce(lsum[:], lp[:], axis=AX.X, op=ALU.add)
        tot_ps2 = ppu.tile([1, 1], F32, tag="pu")
        nc.tensor.matmul(tot_ps2[:], ones_col[:], lsum[:], start=True, stop=True)
        out_sb = spool.tile([1, 1], F32, name="outsb")
        nc.vector.tensor_copy(out_sb[:], tot_ps2[:])
        nc.gpsimd.dma_start(out_d.ap()[:], out_sb[:])

    if postprocess:
        split_excess_waits(nc)
        dedupe_ldweights(nc)
    return nc


_program_cache = {}


def _get_program(has_bias=False):
    key = f"nc{int(has_bias)}"
    if key not in _program_cache:
        _program_cache[key] = build_program(has_bias=has_bias)
    return _program_cache[key]


def make_in_maps(x, W_proj, b_proj, W_qkv, b_qkv, W_out, b_out, W_head, b_head):
    import ml_dtypes

    f8 = np.float64
    w_eff = W_proj.astype(f8) @ W_qkv.astype(f8)  # [32, 768]
    b_eff = b_proj.astype(f8) @ W_qkv.astype(f8) + b_qkv.astype(f8)  # [768]
    Wq, Wk, Wv = w_eff[:, 0:D], w_eff[:, D : 2 * D], w_eff[:, 2 * D : 3 * D]
    bq = b_eff[0:D]
    a_mat = Wq @ Wk.T  # [32, 32]; device computes Y = a_mat^T @ X
    w_m = a_mat @ np.ones(PS)  # [32]: the -r^2 m (A 1).x_s exp-bias term
    w_kb = Wk @ bq  # [32]: the r (Wk bq).x_s exp-bias term (model bias)
    w_oh = W_out.astype(f8) @ W_head.astype(f8)  # [256, 32]
    b_oh = b_out.astype(f8) @ W_head.astype(f8) + b_head.astype(f8)  # [32]
    m_v = Wv @ w_oh  # [32, 32]
    c_v = b_eff[2 * D : 3 * D] @ w_oh + b_oh  # [32]
    mv1 = np.ones(PS) @ m_v  # [32]

    a_b = np.ascontiguousarray(a_mat.astype(ml_dtypes.bfloat16))
    mvg_b = np.ascontiguousarray(
        np.concatenate([m_v, w_m[:, None], w_kb[:, None]], axis=1).astype(
            ml_dtypes.bfloat16
        )
    )
    rows = np.ascontiguousarray(
        np.concatenate([mv1, c_v])[None, :].astype(np.float32)
    )

    # layout-only host prep: patches^T in bf16, and the shifted target
    # patches gathered token-major (zero-padded past the last patch)
    xr = x.reshape(B, T, PS)
    x_t = np.ascontiguousarray(
        xr.transpose(0, 2, 1).astype(ml_dtypes.bfloat16)
    )  # [B, 32, T]
    idx = 1 + np.arange(128)[:, None] + 128 * np.arange(8)[None, :]  # [128, 8]
    x2 = xr[:, np.minimum(idx, T - 1), :]  # [B, 128, 8, 32]
    x2[:, 127, 7, :] = 0.0
    x2 = np.ascontiguousarray(x2.reshape(B, 128, 8 * PS).astype(np.float32))

    in_maps = []
    for core in range(N_CORES):
        sl = slice(core * BPC, (core + 1) * BPC)
        in_maps.append(
            {
                "x_t": np.ascontiguousarray(x_t[sl].reshape(BPC * PS, T)),
                "x2_h": np.ascontiguousarray(x2[sl].reshape(BPC * 128, 8 * PS)),
                "a_mat": a_b,
                "mvg": mvg_b,
                "rows": rows,
            }
        )
    return in_maps


def kernel(**inputs) -> np.ndarray:
    inputs = {k: np.asarray(v) for k, v in inputs.items()}
    has_bias = any(
        float(np.abs(np.asarray(inputs[k])).max()) != 0.0
        for k in ("b_proj", "b_qkv")
    )
    nc = _get_program(has_bias)
    in_maps = make_in_maps(**inputs)
    res = run_bass_kernel_spmd(nc, in_maps, core_ids=list(range(N_CORES)))
    total = sum(float(res.results[i]["loss_partial"][0, 0]) for i in range(N_CORES))
    loss = total / (B * (T - 1) * PS)
    return np.float32(loss)


if __name__ == "__main__":
    rng = np.random.default_rng(0)
    ins = {
        "x": rng.standard_normal((B, L)).astype(np.float32),
        "W_proj": (rng.standard_normal((PS, D)) / math.sqrt(PS)).astype(np.float32),
        "b_proj": np.zeros(D, np.float32),
        "W_qkv": (rng.standard_normal((D, 3 * D)) / math.sqrt(D)).astype(np.float32),
        "b_qkv": np.zeros(3 * D, np.float32),
        "W_out": (rng.standard_normal((D, D)) / math.sqrt(D)).astype(np.float32),
        "b_out": np.zeros(D, np.float32),
        "W_head": (rng.standard_normal((D, PS)) / math.sqrt(D)).astype(np.float32),
        "b_head": np.zeros(PS, np.float32),
    }
    got = kernel(**ins)
    print("kernel loss:", got)


# revision 44
# speedup vs baseline: 1.0108x; 1.0108x over previous
"""Trainium2 Bass kernel for nn_AttentionModel (patch-transformer + MSE loss).

Model (per batch element b of B=32):
    x[b] : [L=32768] --instance-norm--> patches [T=1024, PS=32]
    h    = patches @ W_proj + b_proj                  [T, 256]
    qkv  = h @ W_qkv + b_qkv ;  q,k,v = split(qkv)    [T, 256] each
    attn = softmax(causal(q k^T / 16))                [T, T]
    out  = (attn @ v) @ W_out + b_out                 [T, 256]
    pred = out @ W_head + b_head                      [T, PS]
    loss = mean((pred[:, :-1] - patches[:, 1:])**2)   scalar

Sharding: data-parallel over batch, 4 batch elements per core x 8 cores.
Each core computes a partial sum-of-squares; host combines.

v2.1 design (numpy prototype: proto_v2.py):
  - logits factor through patch space: scores[t,s] = x_t^T A x_s + g_s with
    A = Wq_eff Wk_eff^T [32,32]; the per-t bias constant is dropped
    (softmax-invariant), the per-s part g_s = (Wk b_q)^T x_s rides as a
    per-partition bias of the exp.
  - X = patches^T [32, T] bf16 normalized; Y = A^T X once [32, T];
    scores^T[s,t] computed per s-tile j as ONE wide matmul (N<=1024 bf16),
    causally trimmed.
  - exp is split between ScalarE (ACT exp, per-partition bias) and DVE
    (int16 bit-trick exp writing bf16 bits; its systematic error cancels
    in the softmax ratio -- validated 4.5e-6 rel err in proto).
  - PV runs TRANSPOSED: puT[tc] = sum_j et(j,tc)^T vw_j with
    vw = [X^T M_v + 1 c_v^T | ones], so the softmax denominator lands
    per-PARTITION (puT col 32) and normalize+loss is all per-partition:
    reciprocal [128,8] -> fused (puT*r - xn2) scalar_tensor_tensor ->
    Square+accum loss.
  - the shifted target patches xn2 (= patch[t+1] token-major) come from a
    second strided DMA of x (no transposes, no partition shifts).
  - NOTE row-tiled matmuls (tile_position with row base != 0) pass
    compile+sim but hang this HW stack when >1 issue back-to-back; only
    plain/col tiling is used.
"""

import math
import os

import numpy as np

import concourse.bass as bass
import concourse.mybir as mybir
import concourse.tile as tile
from concourse.bass_utils import run_bass_kernel_spmd
from concourse.masks import make_identity, make_upper_triangular
from concourse.vector_clock import ScopedClock

F32 = mybir.dt.float32
BF16 = mybir.dt.bfloat16
I16 = mybir.dt.int16
AX = mybir.AxisListType
ALU = mybir.AluOpType
AF = mybir.ActivationFunctionType

N_CORES = 8
B = 32
L = 32768
PS = 32
D = 256
T = L // PS  # 1024
BPC = B // N_CORES  # batch elements per core = 4
SCALE = 1.0 / math.sqrt(D)  # 1/16
K1 = 128.0 / math.log(2.0)  # bf16 exponent scale for bit-trick exp
B0 = 127.0 * 128.0 - 0.0579 * 128.0  # bias incl. mantissa-centering corr

# j-tiles whose exp runs on DVE (bit-trick); rest on ScalarE ACT exp.
DVE_EXP = {5, 6, 7}


class SplitDrainTileContext(tile.TileContext):
    """TileContext whose final drain splits sem waits across multiple drain
    instructions -- this walrus rejects >1 sync wait per instruction."""

    def _drain_and_barrier(self, tick_clock, wait_clock):
        probe = mybir.InstDrain(name=f"I-{self.nc.next_id()}", ins=[], outs=[])
        probe.engine = mybir.EngineType.SP
        wait_clock.add_sem_waits(probe, ScopedClock({None: tick_clock.global_clock}))
        waits = list(probe.sync_info.on_wait) if probe.sync_info else []
        assert self.sems is not None
        handles = {h.num: h for h in self.sems.allocated().values()}
        if not waits:
            self.nc.sync.drain()
        for w in waits:
            d = self.nc.sync.drain()
            d.wait_op(handles[w.id], w.wait_value, "sem-ge", check=False)
        self.nc.all_engine_barrier()
        popped = self.nc._tile_sem_poison_stack.pop()
        assert popped is self._sem_poison
        self.nc.clear_and_free_semaphores(list(self.sems.allocated().values()))
        self.nc.all_engine_barrier()


def split_excess_waits(nc, max_waits=1):
    """This walrus rejects instructions carrying more than one sync wait.
    Hoist extra waits onto the immediately preceding same-engine
    instruction when that instruction signals nothing, else insert a
    wait-only drain."""
    for f in nc.m.functions:
        for blk in f.blocks:
            insts = list(blk.instructions)
            out = []
            prev_by_engine = {}
            changed = False
            for inst in insts:
                si = inst.sync_info
                waits = list(si.on_wait) if si else []
                if len(waits) > max_waits:
                    changed = True
                    extra, keep = waits[:-max_waits], waits[-max_waits:]
                    remaining = []
                    prev = prev_by_engine.get(str(inst.engine))
                    for w in extra:
                        psi = prev.sync_info if prev is not None else None
                        if prev is not None and (
                            psi is None
                            or (len(psi.on_wait) == 0 and len(psi.on_update) == 0)
                        ):
                            prev.sync_info = mybir.SyncInfo(on_wait=[w], on_update=[])
                            prev = None  # one hoist per predecessor
                        else:
                            remaining.append(w)
                    for w in remaining:
                        dr = mybir.InstDrain(name=f"I-{nc.next_id()}", ins=[], outs=[])
                        dr.engine = inst.engine
                        dr.sync_info = mybir.SyncInfo(on_wait=[w], on_update=[])
                        out.append(dr)
                    inst.sync_info = mybir.SyncInfo(
                        on_wait=keep, on_update=list(si.on_update)
                    )
                out.append(inst)
                prev_by_engine[str(inst.engine)] = inst
            if changed:
                blk.instructions = out


def dedupe_ldweights(nc):
    """Drop an InstLdweights whose operand AP AND tile_position are
    byte-identical to the immediately preceding PE instruction's
    InstLdweights -- the stationary operand is still loaded. Only legal
    when the elided load carries no sync actions."""
    for f in nc.m.functions:
        for blk in f.blocks:
            insts = list(blk.instructions)
            out = []
            last_pe_ldw_key = None
            changed = False
            for inst in insts:
                if str(inst.engine) != "EngineType.PE":
                    out.append(inst)
                    continue
                tname = type(inst).__name__
                if tname == "InstLdweights":
                    si = inst.sync_info
                    has_sync = si and (len(si.on_wait) or len(si.on_update))
                    try:
                        key = (
                            str(inst.ins[0]),
                            str(getattr(inst, "tile_position", None)),
                        )
                    except Exception:
                        key = None
                    if key is not None and key == last_pe_ldw_key and not has_sync:
                        changed = True
                        continue  # elide duplicate load
                    last_pe_ldw_key = key
                    out.append(inst)
                else:
                    if tname == "InstMatmult":
                        # transpose-mode matmuls reload the array themselves
                        if getattr(inst, "is_transpose", None):
                            last_pe_ldw_key = None
                    else:
                        last_pe_ldw_key = None
                    out.append(inst)
            if changed:
                blk.instructions = out


def build_program(postprocess=True, has_bias=False):
    nc = bass.Bass("TRN2", target_bir_lowering=False, debug=False, num_devices=N_CORES)

    xt_d = nc.dram_tensor("x_t", [BPC * PS, T], BF16, kind="ExternalInput")
    x2_d = nc.dram_tensor("x2_h", [BPC * 128, 8 * PS], F32, kind="ExternalInput")
    a_d = nc.dram_tensor("a_mat", [PS, PS], BF16, kind="ExternalInput")
    mvg_d = nc.dram_tensor("mvg", [PS, PS + 2], BF16, kind="ExternalInput")
    row_d = nc.dram_tensor("rows", [1, 2 * PS], F32, kind="ExternalInput")
    out_d = nc.dram_tensor("loss_partial", [1, 1], F32, kind="ExternalOutput")

    from contextlib import ExitStack

    with SplitDrainTileContext(nc) as tc, ExitStack() as ctx:
        cpool = ctx.enter_context(tc.tile_pool(name="consts", bufs=1))
        # PSUM pools (8 banks x 2KB/partition): pscore 3x2 banks + ppu 2x1
        pscore = ctx.enter_context(tc.tile_pool(name="pscore", bufs=3, space="PSUM"))
        ppu = ctx.enter_context(tc.tile_pool(name="ppu", bufs=2, space="PSUM"))
        # SBUF pools
        xpool = ctx.enter_context(tc.tile_pool(name="xc", bufs=4))
        x2pool = ctx.enter_context(tc.tile_pool(name="xc2", bufs=4))
        qpool = ctx.enter_context(tc.tile_pool(name="xnt", bufs=4))
        ypool = ctx.enter_context(tc.tile_pool(name="y", bufs=3))
        vpool = ctx.enter_context(tc.tile_pool(name="vw", bufs=3))
        epool = ctx.enter_context(tc.tile_pool(name="et", bufs=3))
        spool = ctx.enter_context(tc.tile_pool(name="small", bufs=6))
        lpool = ctx.enter_context(tc.tile_pool(name="loss", bufs=4))

        # ---- constants ----
        ident_f = cpool.tile([128, 128], F32)
        make_identity(nc, ident_f[:])
        triu_b = cpool.tile([128, 128], BF16)
        make_upper_triangular(nc, triu_b[:], val=1.0, diag=True)
        ones_col = cpool.tile([128, 1], F32)
        nc.vector.memset(ones_col[:], 1.0)
        ones_row = cpool.tile([1, 128], F32)
        nc.vector.memset(ones_row[:], 1.0)

        a_mat = cpool.tile([PS, PS], BF16)
        nc.gpsimd.dma_start(a_mat[:], a_d.ap()[:])
        mvg = cpool.tile([PS, PS + 2], BF16)
        nc.gpsimd.dma_start(mvg[:], mvg_d.ap()[:])
        rows = cpool.tile([1, 2 * PS], F32)  # [1^T M_v | c_v]
        nc.gpsimd.dma_start(rows[:], row_d.ap()[:])

        lp = cpool.tile([128, BPC], F32)  # per-batch loss partials
        # last-partition mask: zeroes the t=1023 dd column (excluded by
        # pred[:, :-1]; its xn2 is garbage)
        lmask = cpool.tile([128, 1], F32)
        nc.vector.tensor_scalar(
            out=lmask[:],
            in0=ident_f[:, 127:128],
            scalar1=-1.0,
            scalar2=1.0,
            op0=ALU.mult,
            op1=ALU.add,
        )

        # PE warm-up: HAM holds PE at 1.2 GHz until ~3.4us sustained
        # activity; burn dummy matmuls while the DMAs + stats run.
        warm_ps = pscore.tile([128, 1024], F32, tag="sc")
        for _ in range(60):
            nc.tensor.matmul(
                warm_ps[:, 0:128], triu_b[:], triu_b[:], start=True, stop=True
            )

        # ---- load x_T (patches^T bf16) and shifted target patches ----
        xnt_t = []
        xc2 = []
        for b in range(BPC):
            t_ = qpool.tile([PS, T], BF16, name=f"xnt{b}", tag="xnt")
            nc.sync.dma_start(t_[:], xt_d.ap()[b * PS : (b + 1) * PS, :])
            xnt_t.append(t_)
            t2 = x2pool.tile([128, 8 * PS], F32, name=f"xc2{b}", tag="xc2")
            nc.sync.dma_start(t2[:], x2_d.ap()[b * 128 : (b + 1) * 128, :])
            xc2.append(t2)

        # ---- stats: sums over x2_h (= x minus patch 0, plus zeros) with a
        # patch-0 correction via a second accumulating matmul ----
        sums = spool.tile([128, 2 * BPC], F32, name="sums")
        sqscr = spool.tile([128, 8 * PS], F32, name="sqscr")
        p0sq = spool.tile([PS, 2 * BPC], F32, name="p0sq")
        for b in range(BPC):
            nc.vector.tensor_reduce(
                sums[:, b : b + 1], xc2[b][:], axis=AX.X, op=ALU.add
            )
            nc.scalar.activation(
                sqscr[:],
                xc2[b][:],
                AF.Square,
                accum_out=sums[:, BPC + b : BPC + b + 1],
            )
            p0 = xnt_t[b][:, 0:1]
            nc.vector.tensor_copy(p0sq[:, b : b + 1], p0)
            nc.vector.tensor_tensor(
                out=p0sq[:, BPC + b : BPC + b + 1], in0=p0, in1=p0, op=ALU.mult
            )
        tot_ps = ppu.tile([1, 2 * BPC], F32, tag="pu")
        nc.tensor.matmul(tot_ps[:], ones_col[:], sums[:], start=True, stop=False)
        nc.tensor.matmul(
            tot_ps[:], ones_col[0:PS, :], p0sq[:], start=False, stop=True
        )
        tot = spool.tile([1, 2 * BPC], F32, name="tot")
        nc.vector.tensor_copy(tot[:], tot_ps[:])

        # vectorized stats chain over the 4 batches
        sc = spool.tile([1, 8 * BPC], F32, name="sc")
        mean = sc[:, 0:BPC]
        sm = sc[:, BPC : 2 * BPC]
        varr = sc[:, 2 * BPC : 3 * BPC]
        lnv = sc[:, 3 * BPC : 4 * BPC]
        stde = sc[:, 4 * BPC : 5 * BPC]
        rstd = sc[:, 5 * BPC : 6 * BPC]
        r2 = sc[:, 6 * BPC : 7 * BPC]
        r2m = sc[:, 7 * BPC : 8 * BPC]
        nc.scalar.mul(mean, tot[:, 0:BPC], 1.0 / L)
        nc.vector.tensor_tensor(out=sm, in0=tot[:, 0:BPC], in1=mean, op=ALU.mult)
        nc.vector.tensor_tensor(
            out=varr, in0=tot[:, BPC : 2 * BPC], in1=sm, op=ALU.subtract
        )
        nc.scalar.activation(lnv, varr, AF.Ln, scale=1.0 / (L - 1))
        nc.scalar.activation(stde, lnv, AF.Exp, scale=0.5)  # std
        nc.vector.tensor_scalar_add(stde, stde, 1e-5)
        nc.vector.reciprocal(rstd, stde)  # r
        nc.vector.tensor_tensor(out=r2, in0=rstd, in1=rstd, op=ALU.mult)
        nc.vector.tensor_tensor(out=r2m, in0=r2, in1=mean, op=ALU.mult)
        # broadcast rows: [r^2*K1*SCALE (dve exp mul), r^2*SCALE (act exp
        # scale), -r^2 m*K1*SCALE (dve bias mul), -r^2 m*SCALE (act bias mul)]
        scv = spool.tile([1, 7 * BPC], F32, name="scv")
        nc.vector.tensor_scalar_mul(scv[:, 0:BPC], r2, K1 * SCALE)
        nc.vector.tensor_scalar_mul(scv[:, BPC : 2 * BPC], r2, SCALE)
        nc.vector.tensor_scalar_mul(scv[:, 2 * BPC : 3 * BPC], r2m, -K1 * SCALE)
        nc.vector.tensor_scalar_mul(scv[:, 3 * BPC : 4 * BPC], r2m, -SCALE)
        nc.vector.tensor_scalar_mul(scv[:, 4 * BPC : 5 * BPC], rstd, K1 * SCALE)
        nc.vector.tensor_scalar_mul(scv[:, 5 * BPC : 6 * BPC], rstd, SCALE)
        nc.vector.tensor_copy(scv[:, 6 * BPC : 7 * BPC], r2)
        bc_ps = ppu.tile([128, 7 * BPC], F32, tag="pu")
        nc.tensor.matmul(bc_ps[:], ones_row[:], scv[:], start=True, stop=True)
        bc = spool.tile([128, 7 * BPC], F32, name="bc")
        nc.vector.tensor_copy(bc[:], bc_ps[:])
        # c2row[b] = m_b*(1 - 1^T M_v) + c_v*std_b, replicated to all
        # partitions via a rank-1 matmul
        c2row1 = spool.tile([1, BPC * PS], F32, name="c2row1")
        negm = spool.tile([1, BPC], F32, name="negm")
        nc.vector.tensor_scalar_mul(negm[:], mean, -1.0)
        for b in range(BPC):
            seg = c2row1[:, b * PS : (b + 1) * PS]
            nc.vector.tensor_scalar(
                out=seg,
                in0=rows[:, 0:PS],
                scalar1=negm[:, b : b + 1],
                scalar2=mean[:, b : b + 1],
                op0=ALU.mult,
                op1=ALU.add,
            )
            nc.vector.scalar_tensor_tensor(
                out=seg,
                in0=rows[:, PS : 2 * PS],
                scalar=stde[:, b : b + 1],
                in1=seg,
                op0=ALU.mult,
                op1=ALU.add,
            )
        c2_ps = ppu.tile([128, BPC * PS], F32, tag="pu")
        nc.tensor.matmul(c2_ps[:], ones_row[:], c2row1[:], start=True, stop=True)
        c2rep = spool.tile([128, BPC * PS], F32, name="c2rep")
        nc.vector.tensor_copy(c2rep[:], c2_ps[:])

        ddscr = spool.tile([128, 8 * PS], BF16, name="ddscr")  # Square scratch

        state = {}

        def stage_ty(b):
            xnt = xnt_t[b]
            # Y = A^T X_raw [32, 1024]
            yb = ypool.tile([PS, T], BF16, name=f"y{b}", tag="y")
            y_ps = pscore.tile([128, 1024], F32, tag="sc", name=f"yp{b}")
            for h in range(2):
                nc.tensor.matmul(
                    y_ps[0:PS, 512 * h : 512 * (h + 1)],
                    a_mat[:],
                    xnt[:, 512 * h : 512 * (h + 1)],
                    start=True,
                    stop=True,
                )
            nc.scalar.copy(yb[:, 0:512], y_ps[0:PS, 0:512])
            nc.vector.tensor_copy(yb[:, 512:1024], y_ps[0:PS, 512:1024])
            vw = vpool.tile([128, 8 * (PS + 1)], BF16, name=f"vw{b}", tag="vw")
            nc.gpsimd.memset(
                vw[:].rearrange("u (j e) -> u j e", e=PS + 1)[:, :, PS : PS + 1],
                1.0,
            )
            bias_d = spool.tile([128, 8], F32, name=f"bd{b}", tag="biasd")
            bias_s = spool.tile([128, 8], F32, name=f"bs{b}", tag="biass")
            state[b] = dict(
                xnt=xnt, yb=yb, vw=vw, bias_d=bias_d, bias_s=bias_s
            )

        def stage_scores(b, pv_prev=None):
            st = state[b]
            xnt, yb, vw = st["xnt"], st["yb"], st["vw"]
            vwg_ps = ppu.tile([128, 8 * (PS + 2)], F32, tag="pu", name=f"vwg{b}")
            bias_d, bias_s = st["bias_d"], st["bias_s"]
            # (bias tiles are filled by emit_vw_evac below)
            et = epool.tile([128, 9216], BF16, name=f"et{b}", tag="et")
            st.update(et=et)
            score_tiles = {}

            def emit_scores(j):
                lhs = xnt[:, 128 * j : 128 * (j + 1)]
                nc.tensor.matmul(
                    vwg_ps[:, j * (PS + 2) : (j + 1) * (PS + 2)],
                    lhs,
                    mvg[:],
                    start=True,
                    stop=True,
                )
                s_ps = pscore.tile([128, 1024], F32, tag="sc", name=f"s{b}_{j}")
                score_tiles[j] = s_ps
                for c in range(2):
                    lo = max(512 * c, 128 * j)
                    hi = 512 * (c + 1)
                    if lo >= hi:
                        continue
                    nc.tensor.matmul(
                        s_ps[:, lo:hi], lhs, yb[:, lo:hi], start=True, stop=True
                    )

            def emit_vw_evac(half):
                # cast data cols to bf16; exp bias tiles from the gm/gk cols
                j0 = 4 * half
                pv_v = vwg_ps[
                    :, j0 * (PS + 2) : (j0 + 4) * (PS + 2)
                ].rearrange("u (j e) -> u j e", e=PS + 2)
                vw_v = vw[
                    :, j0 * (PS + 1) : (j0 + 4) * (PS + 1)
                ].rearrange("u (j e) -> u j e", e=PS + 1)
                nc.vector.tensor_copy(vw_v[:, :, 0:PS], pv_v[:, :, 0:PS])
                gm = pv_v[:, :, PS : PS + 1].rearrange("u j o -> u (j o)")
                nc.vector.tensor_scalar(
                    out=bias_d[:, j0 : j0 + 4],
                    in0=gm,
                    scalar1=bc[:, 2 * BPC + b : 2 * BPC + b + 1],
                    scalar2=B0,
                    op0=ALU.mult,
                    op1=ALU.add,
                )
                nc.vector.tensor_scalar(
                    out=bias_s[:, j0 : j0 + 4],
                    in0=gm,
                    scalar1=bc[:, 3 * BPC + b : 3 * BPC + b + 1],
                    scalar2=None,
                    op0=ALU.mult,
                )
                if has_bias:
                    gk = pv_v[:, :, PS + 1 : PS + 2].rearrange("u j o -> u (j o)")
                    nc.vector.scalar_tensor_tensor(
                        out=bias_d[:, j0 : j0 + 4],
                        in0=gk,
                        scalar=bc[:, 4 * BPC + b : 4 * BPC + b + 1],
                        in1=bias_d[:, j0 : j0 + 4],
                        op0=ALU.mult,
                        op1=ALU.add,
                    )
                    nc.vector.scalar_tensor_tensor(
                        out=bias_s[:, j0 : j0 + 4],
                        in0=gk,
                        scalar=bc[:, 5 * BPC + b : 5 * BPC + b + 1],
                        in1=bias_s[:, j0 : j0 + 4],
                        op0=ALU.mult,
                        op1=ALU.add,
                    )

            def emit_exp(j):
                s_ps = score_tiles.pop(j)
                src = s_ps[:, 128 * j : T]
                dst = et[:, 1024 * j + 128 * j : 1024 * (j + 1)]
                if j in DVE_EXP:
                    nc.vector.tensor_scalar(
                        out=dst.bitcast(I16),
                        in0=src,
                        scalar1=bc[:, b : b + 1],
                        scalar2=bias_d[:, j : j + 1],
                        op0=ALU.mult,
                        op1=ALU.add,
                    )
                else:
                    nc.scalar.activation(
                        dst,
                        src,
                        AF.Exp,
                        scale=bc[:, BPC + b : BPC + b + 1],
                        bias=bias_s[:, j : j + 1],
                    )

            def pv_group(tcn):
                if pv_prev is None:
                    return
                pst = state[pv_prev]
                pet, pvw = pst["et"], pst["vw"]
                for j in range(tcn + 1):
                    col = 1024 * j + 128 * tcn
                    nc.tensor.matmul(
                        pst["puT_ps"][:, tcn * (PS + 1) : (tcn + 1) * (PS + 1)],
                        pet[:, col : col + 128],
                        pvw[:, j * (PS + 1) : (j + 1) * (PS + 1)],
                        start=(j == 0),
                        stop=(j == tcn),
                    )

            if pv_prev is not None:
                state[pv_prev]["puT_ps"] = ppu.tile(
                    [128, 8 * (PS + 1)], F32, tag="pu", name=f"puT{pv_prev}"
                )
            for j in range(4):
                emit_scores(j)
                pv_group(j)
            emit_vw_evac(0)
            for j in range(4):
                emit_exp(j)
            for j in range(4, 8):
                emit_scores(j)
                pv_group(j)
            emit_vw_evac(1)
            for j in range(4, 8):
                emit_exp(j)
            # diag triu masks for THIS batch's et (stride 1152), split D/G
            vfull = et[:, 0 : 8 * 1152].rearrange("u (j w) -> u j w", w=1152)[
                :, :, 0:128
            ]
            tri4 = (
                triu_b[:]
                .rearrange("u (o w) -> u o w", o=1)
                .broadcast_to((128, 4, 128))
            )
            nc.vector.tensor_tensor(
                out=vfull[:, 0:4], in0=vfull[:, 0:4], in1=tri4, op=ALU.mult
            )
            nc.gpsimd.tensor_tensor(
                out=vfull[:, 4:8], in0=vfull[:, 4:8], in1=tri4, op=ALU.mult
            )

        def stage_pv_alone(b):
            st = state[b]
            et, vw = st["et"], st["vw"]
            st["puT_ps"] = ppu.tile(
                [128, 8 * (PS + 1)], F32, tag="pu", name=f"puT{b}"
            )
            for tcn in range(8):
                for j in range(tcn + 1):
                    col = 1024 * j + 128 * tcn
                    nc.tensor.matmul(
                        st["puT_ps"][:, tcn * (PS + 1) : (tcn + 1) * (PS + 1)],
                        et[:, col : col + 128],
                        vw[:, j * (PS + 1) : (j + 1) * (PS + 1)],
                        start=(j == 0),
                        stop=(j == tcn),
                    )

        def stage_epi(b):
            st = state[b]
            puT_ps = st["puT_ps"]
            # evacuate puT to SBUF immediately (frees the psum bank), then
            # normalize+subtract+square
            puT_sb = lpool.tile([128, 8 * (PS + 1)], F32, name=f"pus{b}", tag="pus")
            nc.scalar.copy(puT_sb[:], puT_ps[:])
            rcol = lpool.tile([128, 8], F32, name=f"rc{b}", tag="rc")
            nc.vector.reciprocal(
                rcol[:],
                puT_sb[:].rearrange("u (c e) -> u c e", e=PS + 1)[
                    :, :, PS : PS + 1
                ].rearrange("u c o -> u (c o)"),
            )
            rexp = lpool.tile([128, 8 * PS], BF16, name=f"rx{b}", tag="rx")
            nc.vector.tensor_copy(
                rexp[:].rearrange("u (c p) -> u c p", p=PS),
                rcol[:].rearrange("u (c o) -> u c o", o=1).broadcast_to(
                    (128, 8, PS)
                ),
            )
            dd = lpool.tile([128, 8 * PS], BF16, name=f"dd{b}", tag="dd")
            nc.vector.tensor_tensor(
                out=dd[:].rearrange("u (c p) -> u c p", p=PS),
                in0=puT_sb[:].rearrange("u (c e) -> u c e", e=PS + 1)[
                    :, :, 0:PS
                ],
                in1=rexp[:].rearrange("u (c p) -> u c p", p=PS),
                op=ALU.mult,
            )
            nc.gpsimd.tensor_tensor(
                out=dd[:], in0=dd[:], in1=xc2[b][:], op=ALU.subtract
            )
            nc.gpsimd.tensor_tensor(
                out=dd[:].rearrange("u (c p) -> u c p", p=PS),
                in0=dd[:].rearrange("u (c p) -> u c p", p=PS),
                in1=c2rep[:, b * PS : (b + 1) * PS]
                .rearrange("u (o p) -> u o p", o=1)
                .broadcast_to((128, 8, PS)),
                op=ALU.add,
            )
            nc.vector.tensor_scalar(  # exclude t=1023
                out=dd[:, 7 * PS : 8 * PS],
                in0=dd[:, 7 * PS : 8 * PS],
                scalar1=lmask[:],
                scalar2=None,
                op0=ALU.mult,
            )
            nc.scalar.activation(
                ddscr[:], dd[:], AF.Square, accum_out=lp[:, b : b + 1]
            )

        # software pipeline: batch b+1's transposes/Y/scores fill the PE
        # while batch b's exps drain and its PV+epilogue wait on them
        stage_ty(0)
        stage_scores(0)
        for b in range(1, BPC):
            stage_ty(b)
            stage_scores(b, pv_prev=b - 1)
            stage_epi(b - 1)
        stage_pv_alone(BPC - 1)
        stage_epi(BPC - 1)

        # ---- final: scale partials by r^2, then total ----
        nc.vector.tensor_tensor(
            out=lp[:], in0=lp[:], in1=bc[:, 6 * BPC : 7 * BPC], op=ALU.mult
        )
        lsum = spool.tile([128, 1], F32, name="lsum")
        nc.vector.tensor_reduce(lsum[:], lp[:], axis=AX.X, op=ALU.add)
        tot_ps2 = ppu.tile([1, 1], F32, tag="pu")
        nc.tensor.matmul(tot_ps2[:], ones_col[:], lsum[:], start=True, stop=True)
        out_sb = spool.tile([1, 1], F32, name="outsb")
        nc.vector.tensor_copy(out_sb[:], tot_ps2[:])
        nc.gpsimd.dma_start(out_d.ap()[:], out_sb[:])

    if postprocess:
        split_excess_waits(nc)
        dedupe_ldweights(nc)
    return nc


_program_cache = {}


def _get_program(has_bias=False):
    key = f"nc{int(has_bias)}"
    if key not in _program_cache:
        _program_cache[key] = build_program(has_bias=has_bias)
    return _program_cache[key]


def make_in_maps(x, W_proj, b_proj, W_qkv, b_qkv, W_out, b_out, W_head, b_head):
    import ml_dtypes

    f8 = np.float64
    w_eff = W_proj.astype(f8) @ W_qkv.astype(f8)  # [32, 768]
    b_eff = b_proj.astype(f8) @ W_qkv.astype(f8) + b_qkv.astype(f8)  # [768]
    Wq, Wk, Wv = w_eff[:, 0:D], w_eff[:, D : 2 * D], w_eff[:, 2 * D : 3 * D]
    bq = b_eff[0:D]
    a_mat = Wq @ Wk.T  # [32, 32]; device computes Y = a_mat^T @ X
    w_m = a_mat @ np.ones(PS)  # [32]: the -r^2 m (A 1).x_s exp-bias term
    w_kb = Wk @ bq  # [32]: the r (Wk bq).x_s exp-bias term (model bias)
    w_oh = W_out.astype(f8) @ W_head.astype(f8)  # [256, 32]
    b_oh = b_out.astype(f8) @ W_head.astype(f8) + b_head.astype(f8)  # [32]
    m_v = Wv @ w_oh  # [32, 32]
    c_v = b_eff[2 * D : 3 * D] @ w_oh + b_oh  # [32]
    mv1 = np.ones(PS) @ m_v  # [32]

    a_b = np.ascontiguousarray(a_mat.astype(ml_dtypes.bfloat16))
    mvg_b = np.ascontiguousarray(
        np.concatenate([m_v, w_m[:, None], w_kb[:, None]], axis=1).astype(
            ml_dtypes.bfloat16
        )
    )
    rows = np.ascontiguousarray(
        np.concatenate([mv1, c_v])[None, :].astype(np.float32)
    )

    # layout-only host prep: patches^T in bf16, and the shifted target
    # patches gathered token-major (zero-padded past the last patch)
    xr = x.reshape(B, T, PS)
    x_t = np.ascontiguousarray(
        xr.transpose(0, 2, 1).astype(ml_dtypes.bfloat16)
    )  # [B, 32, T]
    idx = 1 + np.arange(128)[:, None] + 128 * np.arange(8)[None, :]  # [128, 8]
    x2 = xr[:, np.minimum(idx, T - 1), :]  # [B, 128, 8, 32]
    x2[:, 127, 7, :] = 0.0
    x2 = np.ascontiguousarray(x2.reshape(B, 128, 8 * PS).astype(np.float32))

    in_maps = []
    for core in range(N_CORES):
        sl = slice(core * BPC, (core + 1) * BPC)
        in_maps.append(
            {
                "x_t": np.ascontiguousarray(x_t[sl].reshape(BPC * PS, T)),
                "x2_h": np.ascontiguousarray(x2[sl].reshape(BPC * 128, 8 * PS)),
                "a_mat": a_b,
                "mvg": mvg_b,
                "rows": rows,
            }
        )
    return in_maps


def kernel(**inputs) -> np.ndarray:
    inputs = {k: np.asarray(v) for k, v in inputs.items()}
    has_bias = any(
        float(np.abs(np.asarray(inputs[k])).max()) != 0.0
        for k in ("b_proj", "b_qkv")
    )
    nc = _get_program(has_bias)
    in_maps = make_in_maps(**inputs)
    res = run_bass_kernel_spmd(nc, in_maps, core_ids=list(range(N_CORES)))
    total = sum(float(res.results[i]["loss_partial"][0, 0]) for i in range(N_CORES))
    loss = total / (B * (T - 1) * PS)
    return np.float32(loss)


if __name__ == "__main__":
    rng = np.random.default_rng(0)
    ins = {
        "x": rng.standard_normal((B, L)).astype(np.float32),
        "W_proj": (rng.standard_normal((PS, D)) / math.sqrt(PS)).astype(np.float32),
        "b_proj": np.zeros(D, np.float32),
        "W_qkv": (rng.standard_normal((D, 3 * D)) / math.sqrt(D)).astype(np.float32),
        "b_qkv": np.zeros(3 * D, np.float32),
        "W_out": (rng.standard_normal((D, D)) / math.sqrt(D)).astype(np.float32),
        "b_out": np.zeros(D, np.float32),
        "W_head": (rng.standard_normal((D, PS)) / math.sqrt(D)).astype(np.float32),
        "b_head": np.zeros(PS, np.float32),
    }
    got = kernel(**ins)
    print("kernel loss:", got)
